# Initial kernel scaffold
#
"""KANAdaptiveFusion Trainium2 kernel (8-core data-parallel).

Math (per reference):
  x = concat(rgb, tactile)                        # (B, 1024)
  base  = silu(x) @ base_weight.T                 # (B, 512)
  bases = cubic B-spline basis of x               # (B, 1024, 8)
  spl   = einsum(bases, spline_weight*scaler)     # (B, 512)
  out   = LayerNorm(base + spl) * gamma + beta

Key identity: with u = 2.5*x + 5.5 (knot-index coordinate, uniform grid
h=0.4 on [-2.2, 2.2]), basis k is the cardinal cubic B-spline
  bases[...,k] = M4(u - k),   6*M4(s) = a^3 - 4*b^3,
  a = relu(2 - |s - 2|),  b = relu(a - 1).
Exact (piecewise cubic), zero outside support -- no clamping needed.

Per-core pipeline (batch-sharded, 2048 rows/core):
  - x^T tiles via PE transpose (fp32)
  - w = Abs(2.5*x^T + (3.5-k))   [ACT]
  - a = Relu(-w + 2)             [ACT]
  - F = a^3 - 4*relu(a-1)^3      [custom DVE op, one pass, bf16 out]
  - spline weights scaled by scaler/6, cast bf16, transposed to
    (k-major, i on partitions) via xbar DMA
  - 72 accumulating matmuls per 128-row tile into PSUM (K chunks =
    8 i-blocks x (8 spline k + 1 silu base))
  - LayerNorm from PSUM (bn_stats/bn_aggr), gamma/beta applied
"""

import sys

if "/opt/trn_rl_repo" not in sys.path:
    sys.path.insert(0, "/opt/trn_rl_repo")

import numpy as np

import concourse.bass as bass
import concourse.mybir as mybir
import concourse.tile as tile
from concourse import bacc
from concourse.bass_utils import run_bass_kernel_spmd
from concourse.masks import make_identity

# ---------------- problem constants (hardcoded per contract) ----------------
B_FULL = 16384
N_CORES = 8
B_CORE = B_FULL // N_CORES  # 2048
IN = 1024
OUT = 512
K8 = 8
IB = IN // 128  # 8 i-blocks
BT = B_CORE // 128  # 16 batch tiles / core
N_GROUPS = 2  # batch groups of 8 tiles (8 PSUM banks)
BT_G = BT // N_GROUPS  # 8
BG = B_CORE // N_GROUPS  # 1024 columns per group
LN_EPS = 1e-5

F32 = mybir.dt.float32
BF16 = mybir.dt.bfloat16

# ---------------- custom DVE op: F = a^3 - imm2 * relu(a-1)^3 ----------------
_M4_OP = None


def _get_m4_op():
    global _M4_OP
    if _M4_OP is not None:
        return _M4_OP
    import concourse.dve_ops as dve_ops
    from concourse.dve_spec import C2, One, Spec, Src0, lower, relu, sq
    from concourse.dve_uop import DveOpSpec

    name = "M4_CUBE_ANT"
    for op in dve_ops.OPS:
        if op.name == name:
            _M4_OP = op
            return op

    def _ref(in0, in1, s0, s1, imm2):
        a = in0.astype(np.float32)
        return a**3 - imm2 * np.maximum(a - 1.0, 0.0) ** 3

    sqa = sq(Src0)
    a3 = sqa * Src0
    b = relu(Src0 - One)
    spec = Spec(body=a3 - sq(b) * (b * C2), reference=_ref)

    row = dve_ops._CUSTOM_DVE_ROW_BASE + len(dve_ops.OPS)
    assert row < 0x20, "custom DVE opcode rows exhausted"

    def _sha(ver):
        return DveOpSpec(
            name=name, opcode=row, uops=lower(spec, ver=ver), rd1_en=False
        ).sha(ver)

    op = dve_ops.DveOp(
        name, spec, subdim=False, uops_sha={"v3": _sha("v3"), "v4": _sha("v4")}
    )
    dve_ops.OPS.append(op)
    dve_ops.CUSTOM_DVE_SPECS[name] = spec
    dve_ops._SUB_OPCODE_FOR_NAME[name] = row
    _M4_OP = op
    return op


# ---------------- per-core Bass program ----------------
_NC_CACHE = None


def _build_nc():
    global _NC_CACHE
    if _NC_CACHE is not None:
        return _NC_CACHE
    m4 = _get_m4_op()
    A = mybir.ActivationFunctionType

    nc = bacc.Bacc("TRN2", target_bir_lowering=False, debug=False, num_devices=N_CORES)

    rgb_d = nc.dram_tensor("rgb_features", (B_CORE, IN // 2), F32, kind="ExternalInput")
    tac_d = nc.dram_tensor(
        "tactile_features", (B_CORE, IN // 2), F32, kind="ExternalInput"
    )
    bw_d = nc.dram_tensor("base_weight", (OUT, IN), F32, kind="ExternalInput")
    sw_d = nc.dram_tensor("spline_weight", (OUT, IN, K8), F32, kind="ExternalInput")
    ss_d = nc.dram_tensor("spline_scaler", (OUT, IN), F32, kind="ExternalInput")
    g_d = nc.dram_tensor("ln_gamma", (OUT,), F32, kind="ExternalInput")
    b_d = nc.dram_tensor("ln_beta", (OUT,), F32, kind="ExternalInput")
    out_d = nc.dram_tensor("out", (B_CORE, OUT), F32, kind="ExternalOutput")

    with tile.TileContext(nc) as tc:
        with (
            tc.tile_pool(name="res", bufs=1) as res,  # resident tensors
            tc.tile_pool(name="prep", bufs=1) as prep,  # weight-prep transients
            tc.tile_pool(name="feat", bufs=2) as feat,  # feature transients
            tc.tile_pool(name="ln", bufs=2) as lnp,  # LN transients
            tc.tile_pool(name="psum", bufs=1, space="PSUM") as pps,
        ):
            # ---- constants ----
            ident = res.tile([128, 128], F32, tag="ident")
            make_identity(nc, ident[:])

            def const_col(val, nm):
                t = res.tile([128, 1], F32, tag=nm, name=nm)
                nc.vector.memset(t[:], val)
                return t

            bias_k = [const_col(3.5 - k, f"biask{k}") for k in range(K8)]
            bias_two = const_col(2.0, "biastwo")
            bias_eps = const_col(LN_EPS, "biaseps")

            # ---- resident: x^T (fp32), spline/base weight tiles (bf16) ----
            xT = [res.tile([128, B_CORE], F32, tag=f"xT{ib}", name=f"xT{ib}") for ib in range(IB)]
            # wT[k][ib]: [128 i, OUT] bf16 ; bwT[ib]: [128 i, OUT] bf16
            wT = [
                [res.tile([128, OUT], BF16, tag=f"wT{k}_{ib}", name=f"wT{k}_{ib}") for ib in range(IB)]
                for k in range(K8)
            ]
            bwT = [res.tile([128, OUT], BF16, tag=f"bwT{ib}", name=f"bwT{ib}") for ib in range(IB)]
            gam_rep = res.tile([128, OUT], F32, tag="gam")
            bet_rep = res.tile([128, OUT], F32, tag="bet")

            # ---- phase A: x^T via PE transpose ----
            for bt in range(BT):
                for ib in range(IB):
                    nat = prep.tile([128, 128], F32, tag="xnat")
                    src = rgb_d if ib < IB // 2 else tac_d
                    icol = (ib % (IB // 2)) * 128
                    nc.sync.dma_start(
                        nat[:],
                        src.ap()[
                            bt * 128 : (bt + 1) * 128, icol : icol + 128
                        ],
                    )
                    tp = pps.tile([128, 128], F32, tag=f"acc{(bt * IB + ib) % 8}", name=f"tp{bt}_{ib}")
                    nc.tensor.transpose(tp[:], nat[:], ident[:])
                    nc.vector.tensor_copy(
                        xT[ib][:, bt * 128 : (bt + 1) * 128], tp[:]
                    )

            # ---- phase B0: gamma/beta broadcast via K=1 matmul ----
            ones1 = res.tile([1, 128], F32, tag="ones1")
            nc.vector.memset(ones1[:], 1.0)
            gsb = prep.tile([1, OUT], F32, tag="gsb")
            bsb = prep.tile([1, OUT], F32, tag="bsb")
            nc.sync.dma_start(gsb[:], g_d.ap().rearrange("(a o) -> a o", a=1))
            nc.sync.dma_start(bsb[:], b_d.ap().rearrange("(a o) -> a o", a=1))
            gps = pps.tile([128, OUT], F32, tag="acc0")
            nc.tensor.matmul(gps[:], ones1[:], gsb[:], start=True, stop=True)
            nc.vector.tensor_copy(gam_rep[:], gps[:])
            bps = pps.tile([128, OUT], F32, tag="acc1")
            nc.tensor.matmul(bps[:], ones1[:], bsb[:], start=True, stop=True)
            nc.vector.tensor_copy(bet_rep[:], bps[:])

            # ---- phase B1: base weight -> bwT (bf16, [i, o]) ----
            for ot in range(OUT // 128):
                bnat = prep.tile([128, IN], F32, tag="bnat")
                nc.sync.dma_start(
                    bnat[:], bw_d.ap()[ot * 128 : (ot + 1) * 128, :]
                )
                bb16 = prep.tile([128, IN], BF16, tag="bb16")
                nc.vector.tensor_copy(bb16[:], bnat[:])
                for ib in range(IB):
                    nc.sync.dma_start_transpose(
                        bwT[ib][:, ot * 128 : (ot + 1) * 128],
                        bb16[:, ib * 128 : (ib + 1) * 128],
                    )

            # ---- phase B2: spline weights -> wT (bf16, k-major, [i, o]) ----
            inv6 = 1.0 / 6.0
            for ot in range(OUT // 128):
                osl = slice(ot * 128, (ot + 1) * 128)
                for ib in range(IB):
                    isl = slice(ib * 128, (ib + 1) * 128)
                    slab = prep.tile([128, 128, K8], F32, tag="slab")
                    nc.sync.dma_start(slab[:], sw_d.ap()[osl, isl, :])
                    ssl = prep.tile([128, 128], F32, tag="ssl")
                    nc.sync.dma_start(ssl[:], ss_d.ap()[osl, isl])
                    # rep[o, k, i] = ssl[o, i] / 6
                    rep = prep.tile([128, K8, 128], F32, tag="rep")
                    for k in range(K8):
                        nc.vector.tensor_scalar_mul(rep[:, k, :], ssl[:], inv6)
                    # sep[o, k, i] = slab[o, i, k] * rep[o, k, i]  (bf16)
                    sep = prep.tile([128, K8, 128], BF16, tag="sep")
                    nc.vector.tensor_mul(
                        sep[:], slab[:].rearrange("p i k -> p k i"), rep[:]
                    )
                    for k in range(K8):
                        nc.sync.dma_start_transpose(
                            wT[k][ib][:, osl], sep[:, k, :]
                        )

            # ---- phase C: features + matmuls + LN, per batch group ----
            for g in range(N_GROUPS):
                bsl = slice(g * BG, (g + 1) * BG)
                acc = [
                    pps.tile([128, OUT], F32, tag=f"acc{t}", name=f"acc_g{g}_t{t}") for t in range(BT_G)
                ]
                for ib in range(IB):
                    xs = xT[ib][:, bsl]
                    sil = feat.tile([128, BG], BF16, tag="sil")
                    nc.scalar.activation(sil[:], xs, A.Silu)
                    for t in range(BT_G):
                        nc.tensor.matmul(
                            acc[t][:],
                            sil[:, t * 128 : (t + 1) * 128],
                            bwT[ib][:],
                            start=(ib == 0),
                            stop=False,
                        )
                    for k in range(K8):
                        w = feat.tile([128, BG], F32, tag="w")
                        nc.scalar.activation(
                            w[:], xs, A.Abs, bias=bias_k[k][:], scale=2.5
                        )
                        nc.scalar.activation(
                            w[:], w[:], A.Relu, bias=bias_two[:], scale=-1.0
                        )
                        F = feat.tile([128, BG], BF16, tag="F")
                        nc.vector._custom_dve(m4, out=F[:], in0=w[:], imm2=4.0)
                        last = ib == IB - 1 and k == K8 - 1
                        for t in range(BT_G):
                            nc.tensor.matmul(
                                acc[t][:],
                                F[:, t * 128 : (t + 1) * 128],
                                wT[k][ib][:],
                                start=False,
                                stop=last,
                            )
                # LayerNorm + gamma/beta + store
                for t in range(BT_G):
                    stats = lnp.tile([128, 6], F32, tag="stats")
                    nc.vector.bn_stats(stats[:], acc[t][:])
                    mv = lnp.tile([128, 2], F32, tag="mv")
                    nc.vector.bn_aggr(mv[:], stats[:])
                    rstd = lnp.tile([128, 1], F32, tag="rstd")
                    nc.scalar.activation(
                        rstd[:], mv[:, 1:2], A.Sqrt, bias=bias_eps[:]
                    )
                    nc.vector.reciprocal(rstd[:], rstd[:])
                    nrm = lnp.tile([128, OUT], F32, tag="nrm")
                    nc.vector.tensor_scalar(
                        out=nrm[:],
                        in0=acc[t][:],
                        scalar1=mv[:, 0:1],
                        scalar2=rstd[:],
                        op0=mybir.AluOpType.subtract,
                        op1=mybir.AluOpType.mult,
                    )
                    nc.vector.tensor_mul(nrm[:], nrm[:], gam_rep[:])
                    nc.vector.tensor_add(nrm[:], nrm[:], bet_rep[:])
                    row = g * BG + t * 128
                    nc.sync.dma_start(out_d.ap()[row : row + 128, :], nrm[:])

    nc.compile()
    _NC_CACHE = nc
    return nc


# ---------------- public entry point ----------------
def kernel(
    rgb_features,
    tactile_features,
    base_weight,
    spline_weight,
    spline_scaler,
    ln_gamma,
    ln_beta,
    **_unused,
):
    rgb = np.ascontiguousarray(np.asarray(rgb_features, dtype=np.float32))
    tac = np.ascontiguousarray(np.asarray(tactile_features, dtype=np.float32))
    shared = {
        "base_weight": np.ascontiguousarray(np.asarray(base_weight, np.float32)),
        "spline_weight": np.ascontiguousarray(np.asarray(spline_weight, np.float32)),
        "spline_scaler": np.ascontiguousarray(np.asarray(spline_scaler, np.float32)),
        "ln_gamma": np.ascontiguousarray(np.asarray(ln_gamma, np.float32)),
        "ln_beta": np.ascontiguousarray(np.asarray(ln_beta, np.float32)),
    }
    nc = _build_nc()
    in_maps = []
    for c in range(N_CORES):
        sl = slice(c * B_CORE, (c + 1) * B_CORE)
        m = {"rgb_features": rgb[sl], "tactile_features": tac[sl]}
        m.update(shared)
        in_maps.append(m)
    res = run_bass_kernel_spmd(nc, in_maps, core_ids=list(range(N_CORES)))
    return np.concatenate([res.results[c]["out"] for c in range(N_CORES)], axis=0)


if __name__ == "__main__":
    xs = np.random.randn(B_FULL, IN // 2).astype(np.float32)
    out = kernel(
        rgb_features=xs,
        tactile_features=xs,
        base_weight=np.random.randn(OUT, IN).astype(np.float32) * 0.02,
        spline_weight=np.random.randn(OUT, IN, K8).astype(np.float32) * 0.02,
        spline_scaler=np.ones((OUT, IN), np.float32),
        ln_gamma=np.ones((OUT,), np.float32),
        ln_beta=np.zeros((OUT,), np.float32),
    )
    print(out.shape, out.dtype)



# revision 17
# speedup vs baseline: 2.9068x; 2.9068x over previous
"""KANAdaptiveFusion Trainium2 kernel (8-core data-parallel), v3.

Math (per reference):
  x = concat(rgb, tactile)                        # (B, 1024)
  base  = silu(x) @ base_weight.T                 # (B, 512)
  bases = cubic B-spline basis of x               # (B, 1024, 8)
  spl   = einsum(bases, spline_weight*scaler)     # (B, 512)
  out   = LayerNorm(base + spl) * gamma + beta

Key identity: basis k is the cardinal cubic B-spline
  6*M4(u-k) = a^3 - 4*b^3,  a = relu(2 - |2.5x + 3.5-k|), b = relu(a-1).
We compute F = 6*M4 (no /6) and scale the base path by 6; LayerNorm is
scale-invariant (the eps shift is ~2e-5 relative, negligible).

v3 design (per core, 2048 rows):
  - x -> bf16 (DVE cast) -> batched xbar DMA transpose (one instruction
    per batch tile, 8 sub-tiles each) into xT [i, bt, ib-packed]
  - weights: slab f32 * scaler (gpsimd broadcast mul, bf16 k-major)
    -> batched xbar transpose into wT[ib] [i, ot, k, o]
  - F per (group, ib): 6 k's on ACT (Abs f32 -> Relu bf16), 2 k's via a
    6-uop custom DVE op; cubes via the 8-uop M4 DVE op on k-pairs
  - 1152 bf16 matmuls [128x128]x[128x512] accumulate in PSUM; 2 batch
    groups x 8 tiles use all 8 PSUM banks
  - LN from PSUM: bn_stats/bn_aggr, rstd via Abs_reciprocal_sqrt batched
    per group (one ACT table-set switch each way), gamma/beta on gpsimd
"""

import sys

if "/opt/trn_rl_repo" not in sys.path:
    sys.path.insert(0, "/opt/trn_rl_repo")

import numpy as np

import concourse.bass as bass
import concourse.mybir as mybir
import concourse.tile as tile
from concourse import bacc
from concourse.bass_utils import run_bass_kernel_spmd
from concourse.masks import make_identity

# ---------------- problem constants (hardcoded per contract) ----------------
B_FULL = 16384
N_CORES = 8
B_CORE = B_FULL // N_CORES  # 2048
IN = 1024
OUT = 512
K8 = 8
IB = IN // 128  # 8 i-blocks
OT4 = OUT // 128  # 4 o-blocks
BT = B_CORE // 128  # 16 batch tiles
N_GROUPS = 2
BT_G = BT // N_GROUPS  # 8 batch tiles per group
BG = B_CORE // N_GROUPS  # 1024 columns per group
LN_EPS = 1e-5

ACT_KS = (0, 1, 2, 3)  # a produced on the scalar engine (Abs+Relu)

F32 = mybir.dt.float32
BF16 = mybir.dt.bfloat16


def _cprime(k):
    return (3.5 - k) / 2.5


# ---------------- custom DVE ops ----------------
_M4_OP = None
_A_OP = None


def _get_m4_op():
    """F = a^3 - imm2 * relu(a-1)^3   (8 uops, from a)."""
    global _M4_OP
    if _M4_OP is not None:
        return _M4_OP
    import concourse.dve_ops as dve_ops
    from concourse.dve_spec import C2, One, Spec, Src0, relu, sq

    name = "M4_CUBE_ANT"
    for op in dve_ops.OPS:
        if op.name == name:
            _M4_OP = op
            return op

    def _ref(in0, in1, s0, s1, imm2):
        a = in0.astype(np.float32)
        return a**3 - imm2 * np.maximum(a - 1.0, 0.0) ** 3

    sqa = sq(Src0)
    a3 = sqa * Src0
    b = relu(Src0 - One)
    spec = Spec(body=a3 - sq(b) * (b * C2), reference=_ref)
    _M4_OP = _register_op(name, spec)
    return _M4_OP


def _get_a_op():
    """a = relu(imm2 - s1*|x + s0|)   (6 uops, from x)."""
    global _A_OP
    if _A_OP is not None:
        return _A_OP
    import concourse.dve_ops as dve_ops
    from concourse.dve_spec import C0, C1, C2, Spec, Src0, Zero, maxx, relu

    name = "M4_A_ANT"
    for op in dve_ops.OPS:
        if op.name == name:
            _A_OP = op
            return op

    def _ref(in0, in1, s0, s1, imm2):
        x = in0.astype(np.float32)
        return np.maximum(imm2 - s1 * np.abs(x + s0), 0.0)

    t = Src0 + C0
    w = maxx(t, Zero - t)
    spec = Spec(body=relu(C2 - w * C1), reference=_ref)
    _A_OP = _register_op(name, spec)
    return _A_OP


def _register_op(name, spec):
    import concourse.dve_ops as dve_ops
    from concourse.dve_spec import lower
    from concourse.dve_uop import DveOpSpec

    row = dve_ops._CUSTOM_DVE_ROW_BASE + len(dve_ops.OPS)
    assert row < 0x20, "custom DVE opcode rows exhausted"

    def _sha(ver):
        return DveOpSpec(
            name=name, opcode=row, uops=lower(spec, ver=ver), rd1_en=False
        ).sha(ver)

    op = dve_ops.DveOp(
        name, spec, subdim=False, uops_sha={"v3": _sha("v3"), "v4": _sha("v4")}
    )
    dve_ops.OPS.append(op)
    dve_ops.CUSTOM_DVE_SPECS[name] = spec
    dve_ops._SUB_OPCODE_FOR_NAME[name] = row
    return op


# ---------------- per-core Bass program ----------------
_NC_CACHE = {}


def _build_nc(apply_affine=False):
    if apply_affine in _NC_CACHE:
        return _NC_CACHE[apply_affine]
    m4 = _get_m4_op()
    aop = _get_a_op()
    A = mybir.ActivationFunctionType

    nc = bacc.Bacc("TRN2", target_bir_lowering=False, debug=False, num_devices=N_CORES)

    rgb_d = nc.dram_tensor("rgb_features", (B_CORE, IN // 2), F32, kind="ExternalInput")
    tac_d = nc.dram_tensor(
        "tactile_features", (B_CORE, IN // 2), F32, kind="ExternalInput"
    )
    bw_d = nc.dram_tensor("base_weight", (OUT, IN), F32, kind="ExternalInput")
    sw_d = nc.dram_tensor("spline_weight", (OUT, IN, K8), F32, kind="ExternalInput")
    ss_d = nc.dram_tensor("spline_scaler", (OUT, IN), F32, kind="ExternalInput")
    g_d = nc.dram_tensor("ln_gamma", (OUT,), F32, kind="ExternalInput")
    b_d = nc.dram_tensor("ln_beta", (OUT,), F32, kind="ExternalInput")
    out_d = nc.dram_tensor("out", (B_CORE, OUT), F32, kind="ExternalOutput")

    with tile.TileContext(nc) as tc:
        with (
            tc.tile_pool(name="res", bufs=1) as res,  # resident tensors
            tc.tile_pool(name="xp", bufs=1) as xp,  # x staging
            tc.tile_pool(name="wp", bufs=2) as wp,  # weight staging
            tc.tile_pool(name="feat", bufs=2) as feat,  # F transients
            tc.tile_pool(name="ln", bufs=2) as lnp,  # LN transients
            tc.tile_pool(name="psum", bufs=1, space="PSUM") as pps,
        ):
            # ---- constants ----
            def const_col(val, nm):
                t = res.tile([128, 1], F32, tag=nm, name=nm)
                nc.vector.memset(t[:], val)
                return t

            bias_k = {k: const_col(_cprime(k), f"biask{k}") for k in ACT_KS}
            ident = res.tile([128, 128], F32, tag="ident")
            make_identity(nc, ident[:])
            bias_two = const_col(2.0, "biastwo")
            bias_eps = const_col(LN_EPS, "biaseps")

            # ---- resident tensors (batched-transpose friendly layouts) ----
            # xT[i-part, bt, ib, b]  -- one xbar transpose per batch tile
            xTa = res.tile([128, BT, IB, 128], BF16, tag="xTa", name="xTa")
            # wT[ib][i-part, ot, k, o]  -- one xbar transpose per (ib, ot)
            wTa = [
                res.tile([128, OT4, K8, 128], BF16, tag=f"wTa{ib}", name=f"wTa{ib}")
                for ib in range(IB)
            ]
            # bwT[i-part, ot, ib, o]  -- one xbar transpose per ot
            bwTa = res.tile([128, OT4, IB, 128], BF16, tag="bwTa", name="bwTa")
            gam_rep = res.tile([128, OUT], F32, tag="gam")
            bet_rep = res.tile([128, OUT], F32, tag="bet")

            def xs_ap(g, ib):
                # [128, 8 bt, 128 b] strided view of group g's columns
                return xTa[:, g * BT_G : (g + 1) * BT_G, ib, :]

            def wmov(k, ib):
                # moving operand [128 i, 4 ot, 128 o] (free 512)
                return wTa[ib][:, :, k, :]

            def bwmov(ib):
                return bwTa[:, :, ib, :]

            def emit_gamma_beta():
                ones1 = res.tile([1, 128], F32, tag="ones1")
                nc.vector.memset(ones1[:], 1.0)
                gsb = wp.tile([1, OUT], F32, tag="gsb")
                bsb = wp.tile([1, OUT], F32, tag="bsb")
                nc.sync.dma_start(gsb[:], g_d.ap().rearrange("(a o) -> a o", a=1))
                nc.sync.dma_start(bsb[:], b_d.ap().rearrange("(a o) -> a o", a=1))
                gps = pps.tile([128, OUT], F32, tag="acc6", name="gps")
                nc.tensor.matmul(gps[:], ones1[:], gsb[:], start=True, stop=True)
                nc.vector.tensor_copy(gam_rep[:], gps[:])
                bps = pps.tile([128, OUT], F32, tag="acc7", name="bps")
                nc.tensor.matmul(bps[:], ones1[:], bsb[:], start=True, stop=True)
                nc.vector.tensor_copy(bet_rep[:], bps[:])

            # ---------------- emission helpers ----------------
            x16_tiles = {}

            def phase_x_loads(g):
                """Group g's x rows, one pair (2 batch tiles) per DMA:
                rgb on sync, tac on scalar, DVE casts to bf16 (bt-major),
                double-buffered rings so loads overlap casts."""
                for q in range(4):
                    r0 = (g * BT_G + q * 2) * 128
                    rgb2 = xp.tile(
                        [128, 2, IN // 2], F32, tag="rgb2", bufs=2,
                        name=f"rgb2_{g}_{q}",
                    )
                    nc.sync.dma_start(
                        rgb2[:],
                        rgb_d.ap()[r0 : r0 + 256, :].rearrange(
                            "(j p) c -> p j c", j=2
                        ),
                    )
                    tac2 = xp.tile(
                        [128, 2, IN // 2], F32, tag="tac2", bufs=2,
                        name=f"tac2_{g}_{q}",
                    )
                    nc.scalar.dma_start(
                        tac2[:],
                        tac_d.ap()[r0 : r0 + 256, :].rearrange(
                            "(j p) c -> p j c", j=2
                        ),
                    )
                    if g == 0:
                        # group 0: PE transposes read the f32 tiles directly
                        x16_tiles[(g, q)] = (rgb2, tac2)
                    else:
                        x16p = xp.tile(
                            [128, 2, IN], BF16, tag="x16p", bufs=2,
                            name=f"x16p_{g}_{q}",
                        )
                        nc.vector.tensor_copy(x16p[:, :, : IN // 2], rgb2[:])
                        nc.vector.tensor_copy(x16p[:, :, IN // 2 :], tac2[:])
                        x16_tiles[(g, q)] = x16p

            def phase_x_T(g):
                """Group 1: batched xbar transposes (sync queue, mid-run)."""
                for q in range(4):
                    bt0 = g * BT_G + q * 2
                    nc.sync.dma_start_transpose(
                        xTa[:, bt0 : bt0 + 2, :, :], x16_tiles.pop((g, q))[:]
                    )

            def phase_x_T_pe():
                """Group 0: transpose on the (startup-idle) PE in f32,
                flush-cast from PSUM to bf16 xTa on the (startup-idle) ACT.
                PSUM staging borrows the acc6/acc7 banks, which group 0's
                accumulation does not touch until these flushes are read."""
                A_ = mybir.ActivationFunctionType
                for q in range(4):
                    rgb2, tac2 = x16_tiles.pop((0, q))
                    for tt in range(2):
                        bt = q * 2 + tt
                        for half in range(2):
                            stage = pps.tile(
                                [128, 512], F32, tag=f"acc{6 + (bt + half) % 2}",
                                name=f"xst{bt}_{half}",
                            )
                            for j in range(4):
                                ib = half * 4 + j
                                src = rgb2 if ib < 4 else tac2
                                icol = (ib % 4) * 128
                                nc.tensor.transpose(
                                    stage[:, j * 128 : (j + 1) * 128],
                                    src[:, tt, icol : icol + 128],
                                    ident[:],
                                )
                            nc.scalar.activation(
                                xTa[:, bt, half * 4 : half * 4 + 4, :],
                                stage[:],
                                A_.Copy,
                                scale=1.0,
                            )

            c16_tiles = {}

            def phase_w_loads(ib):
                """Load spline weight slabs for i-block ib, fused scaler mul
                + bf16 cast on gpsimd (k-major)."""
                isl = slice(ib * 128, (ib + 1) * 128)
                ssl4 = wp.tile([128, OT4, 128], F32, tag="ssl4", name=f"ssl4_{ib}")
                nc.sync.dma_start(
                    ssl4[:],
                    ss_d.ap()[:, isl].rearrange("(j p) i -> p j i", j=OT4),
                )
                for ot in range(OT4):
                    osl = slice(ot * 128, (ot + 1) * 128)
                    slab = wp.tile(
                        [128, 128, K8], F32, tag="slab", bufs=4, name=f"slab{ib}_{ot}"
                    )
                    nc.sync.dma_start(slab[:], sw_d.ap()[osl, isl, :])
                    c16 = wp.tile(
                        [128, K8, 128], BF16, tag="c16", bufs=4, name=f"c16_{ib}_{ot}"
                    )
                    nc.gpsimd.tensor_mul(
                        c16[:],
                        slab[:].rearrange("p i k -> p k i"),
                        ssl4[:, ot, :].unsqueeze(1).broadcast_to((128, K8, 128)),
                    )
                    c16_tiles[(ib, ot)] = c16

            def phase_w_T(ib):
                for ot in range(OT4):
                    nc.sync.dma_start_transpose(
                        wTa[ib][:, ot, :, :], c16_tiles.pop((ib, ot))[:]
                    )

            def phase_bw():
                """Base weights: load, scale by 6 (LN-invariance trick), cast,
                one batched xbar transpose per ot into bwTa."""
                for ot in range(OT4):
                    osl = slice(ot * 128, (ot + 1) * 128)
                    bnat = wp.tile(
                        [128, IN], F32, tag="slab", bufs=4, name=f"bnat{ot}"
                    )
                    nc.sync.dma_start(bnat[:], bw_d.ap()[osl, :])
                    b16 = wp.tile([128, IN], BF16, tag="c16", bufs=4, name=f"b16_{ot}")
                    nc.vector.tensor_scalar_mul(b16[:], bnat[:], 6.0)
                    nc.sync.dma_start_transpose(bwTa[:, ot, :, :], b16[:])

            def emit_f_and_matmuls(g, ib, accs, sils, pairs=None):
                """F production + spline matmuls for (group g, i-block ib)."""
                xs = xs_ap(g, ib)
                if pairs is None:
                    pairs = range(K8 // 2)
                for kp in pairs:
                    apair = feat.tile(
                        [128, 2, BG], BF16, tag="apair", bufs=2, name=f"ap{g}_{ib}_{kp}"
                    )
                    for kk in range(2):
                        k = 2 * kp + kk
                        if k in ACT_KS:
                            w = feat.tile(
                                [128, BG], F32, tag="w", name=f"w{g}_{ib}_{k}"
                            )
                            nc.scalar.activation(
                                w[:], xs, A.Abs, bias=bias_k[k][:], scale=1.0
                            )
                            nc.scalar.activation(
                                apair[:, kk, :],
                                w[:],
                                A.Relu,
                                bias=bias_two[:],
                                scale=-2.5,
                            )
                        else:
                            nc.vector._custom_dve(
                                aop,
                                out=apair[:, kk, :],
                                in0=xs,
                                s0=_cprime(k),
                                s1=2.5,
                                imm2=2.0,
                            )
                    Fp = feat.tile(
                        [128, 2, BG], BF16, tag="Fp", bufs=2, name=f"F{g}_{ib}_{kp}"
                    )
                    nc.vector._custom_dve(m4, out=Fp[:], in0=apair[:], imm2=4.0)
                    for kk in range(2):
                        k = 2 * kp + kk
                        for t in range(BT_G):
                            nc.tensor.matmul(
                                accs[t][:],
                                Fp[:, kk, t * 128 : (t + 1) * 128],
                                wmov(k, ib),
                                start=(ib == 0 and k == 0),
                                stop=False,
                            )
                if (K8 // 2 - 1) in pairs:
                    sil = feat.tile(
                        [128, BG], BF16, tag=f"sil{ib}", bufs=1, name=f"sil{g}_{ib}"
                    )
                    nc.scalar.activation(sil[:], xs, A.Silu, scale=1.0)
                    sils.append(sil)

            def emit_silu_matmuls(accs, sils):
                # t-major so each PSUM bank finishes (and LayerNorm can
                # drain it) while later banks still accumulate
                for t in range(BT_G):
                    for ib in range(IB):
                        nc.tensor.matmul(
                            accs[t][:],
                            sils[ib][:, t * 128 : (t + 1) * 128],
                            bwmov(ib),
                            start=False,
                            stop=(ib == IB - 1),
                        )

            def phase_ln(g, accs):
                """LayerNorm + gamma/beta + store, per tile so each PSUM
                bank drains as soon as its accumulation finishes. Abs/Relu
                live in every ACT table set, so the sqrt-set is entered once
                per group (next Silu switches back)."""
                for t in range(BT_G):
                    stats = lnp.tile([128, 6], F32, tag="stats", name=f"st{g}_{t}")
                    nc.vector.bn_stats(stats[:], accs[t][:])
                    mv = lnp.tile([128, 2], F32, tag="mv", bufs=8, name=f"mv{g}_{t}")
                    nc.vector.bn_aggr(mv[:], stats[:])
                    rstd = lnp.tile([128, 1], F32, tag="rstd", bufs=8, name=f"rs{g}_{t}")
                    nc.scalar.activation(
                        rstd[:], mv[:, 1:2], A.Abs_reciprocal_sqrt, bias=bias_eps[:]
                    )
                    nrm = lnp.tile([128, OUT], F32, tag="nrm", name=f"nrm{g}_{t}")
                    nc.vector.tensor_scalar(
                        out=nrm[:],
                        in0=accs[t][:],
                        scalar1=mv[:, 0:1],
                        scalar2=rstd[:],
                        op0=mybir.AluOpType.subtract,
                        op1=mybir.AluOpType.mult,
                    )
                    if apply_affine:
                        nc.gpsimd.tensor_mul(nrm[:], nrm[:], gam_rep[:])
                        nc.gpsimd.tensor_add(nrm[:], nrm[:], bet_rep[:])
                    row = g * BG + t * 128
                    nc.sync.dma_start(out_d.ap()[row : row + 128, :], nrm[:])

            # ---------------- emission schedule ----------------
            def group_accs(g):
                return [
                    pps.tile([128, OUT], F32, tag=f"acc{t}", name=f"acc{g}_{t}")
                    for t in range(BT_G)
                ]

            phase_w_loads(0)
            phase_x_loads(0)
            phase_x_T_pe()
            phase_w_T(0)
            if apply_affine:
                emit_gamma_beta()
            accs0 = group_accs(0)
            sils0 = []
            for ib in range(IB):
                if ib + 1 < IB:
                    phase_w_loads(ib + 1)
                emit_f_and_matmuls(0, ib, accs0, sils0)
                if ib == 0:
                    phase_x_loads(1)
                    phase_x_T(1)
                if ib + 1 < IB:
                    phase_w_T(ib + 1)
                if ib == 5:
                    phase_bw()
            emit_silu_matmuls(accs0, sils0)
            accs1 = group_accs(1)
            sils1 = []
            # two pairs of group-1 F work ahead of LN(0): they fill the
            # silu-matmul window; more would WAR-cycle through the Fp ring
            emit_f_and_matmuls(1, 0, accs1, sils1, pairs=(0, 1))
            phase_ln(0, accs0)
            emit_f_and_matmuls(1, 0, accs1, sils1, pairs=(2, 3))
            for ib in range(1, IB):
                emit_f_and_matmuls(1, ib, accs1, sils1)
            emit_silu_matmuls(accs1, sils1)
            phase_ln(1, accs1)

    nc.compile()
    _NC_CACHE[apply_affine] = nc
    return nc


# ---------------- public entry point ----------------
def _core_inputs(inputs, c):
    sl = slice(c * B_CORE, (c + 1) * B_CORE)
    m = {
        "rgb_features": np.ascontiguousarray(
            np.asarray(inputs["rgb_features"], np.float32)[sl]
        ),
        "tactile_features": np.ascontiguousarray(
            np.asarray(inputs["tactile_features"], np.float32)[sl]
        ),
    }
    for k in ("base_weight", "spline_weight", "spline_scaler", "ln_gamma", "ln_beta"):
        m[k] = np.ascontiguousarray(np.asarray(inputs[k], np.float32))
    return m


def _gather(res):
    return np.concatenate(
        [res.results[c]["out"] for c in range(N_CORES)], axis=0
    )


def _needs_affine(inputs):
    g = np.asarray(inputs["ln_gamma"], np.float32)
    b = np.asarray(inputs["ln_beta"], np.float32)
    return not (np.all(g == 1.0) and np.all(b == 0.0))


def kernel(**inputs):
    nc = _build_nc(apply_affine=_needs_affine(inputs))
    in_maps = [_core_inputs(inputs, c) for c in range(N_CORES)]
    res = run_bass_kernel_spmd(nc, in_maps, core_ids=list(range(N_CORES)))
    return _gather(res)


if __name__ == "__main__":
    xs = np.random.randn(B_FULL, IN // 2).astype(np.float32)
    out = kernel(
        rgb_features=xs,
        tactile_features=xs,
        base_weight=np.random.randn(OUT, IN).astype(np.float32) * 0.02,
        spline_weight=np.random.randn(OUT, IN, K8).astype(np.float32) * 0.02,
        spline_scaler=np.ones((OUT, IN), np.float32),
        ln_gamma=np.ones((OUT,), np.float32),
        ln_beta=np.zeros((OUT,), np.float32),
    )
    print(out.shape, out.dtype)
